# revision 42
# baseline (speedup 1.0000x reference)
"""Trainium2 Bass kernel for the expert-choice MoE layer (nn_MoELayer_18451179504170).

Expert-parallel across 8 NeuronCores (2 experts/core), with a
collective-sharded router.  Per core:

  ROUTER (sharded, full fp32 precision): each core loads X^T for only its
  1024 tokens (4MB) and computes silu(X@R1)@R2 for all 16 experts with plain
  fp32 matmuls.  Full fp32 matters: the expert-choice top-256 boundary gaps
  get as small as ~2e-5, and fp32r's ~1.5e-4 logit noise swaps several
  boundary tokens (measured 1.9e-2 rel err vs the 2e-2 gate).  An AllToAll
  (64KB/core, ~17us) redistributes logits so each core holds its 2 experts'
  rows for all 8192 tokens.  A throwaway-matmul warm-up burns the initial
  DMA wait so the router runs at full PE clock (the tensor engine ramps
  0.65 -> 2.4 GHz only after ~3us of continuous work).

  TOPK/DISPATCH, per (expert, batch): lane-wrapped logit views load in ONE
  strided DMA per (expert, kind) on the Act queue (partition = column
  group, row-half in the free dim, so every view dim maps to one source
  dim); gpsimd.kth_largest yields the exact 257th-largest logit as the
  selection threshold; mask -> iota/copy_predicated -> sparse_gather
  compacts the 256 winner ids (the exp-weight bookkeeping for the gate
  normalization is deferred off this critical path); a tiny matmul
  replicates ids to 128 partitions; and dma_gather(transpose=True) pulls
  the 256 token rows from a bf16 copy of the activations DIRECTLY into the
  [d-part, k, token] layout the expert matmuls consume.  Chain-major
  emission (kth-b0 -> sel-b0 -> kth-b1 -> gather-b0 -> ...) gets batch-0's
  gather issued as early as possible.  w3 loads are dep-free, so the Tile
  scheduler would hoist them into this window and starve the gathers of
  DMA bandwidth -- tile_wait_until floors them past it.

  EXPERTS (PE-bound, 327.7us theory floor of the ~388us total): mm1/mm2
  in bf16 (stationary W1/W2 chunks streamed through one cross-expert tag
  ring, moving gathered tokens, fp32 PSUM accumulation; the two 256-col
  token blocks sharing a PSUM bank form one accumulation group since
  `start` zeroes the whole 2KB bank).  Batch-major matmul order plus an
  m0/m1-interleaved first iteration give each arriving gather a 24-matmul
  runway, matching the Pool engine's per-chain dispatch cadence.  ONE
  cross-phase/cross-expert PSUM tag ring (bufs=3 of [128,1024] = 6 banks)
  serves mm1-g, mm2-v and the mm3-o tiles, so phase and expert transitions
  never wait on a pool handoff.  h and W3 are bf16 (mm3 stays 1 cyc/row,
  w3 DMA bytes halve); the gate weight applies per 512-col PSUM-bank half
  during the drain on alternating DVE/Act engines, each half scattering
  independently (elem_size=512, elem_step=1024) so only a half-row chain
  is exposed after the final matmul.  End-to-end 4.2e-3 rel L2 (selection
  stays fp32-exact).

Host side: slices per-core X^T shards and per-expert weights, makes the
bf16 activation copy, and sums the 8 partial outputs.
"""

import contextlib
import os
import sys

for _p in ("/opt/trn_rl_repo", "/root/.axon_site/_ro/trn_rl_repo"):
    if os.path.isdir(_p) and _p not in sys.path:
        sys.path.insert(0, _p)

import numpy as np

import concourse.bass as bass  # noqa: F401
import concourse.mybir as mybir
from concourse import bacc
from concourse.tile import TileContext
from concourse.bass_utils import run_bass_kernel_spmd

F32 = mybir.dt.float32
F32R = mybir.dt.float32r
BF16 = mybir.dt.bfloat16
AF = mybir.ActivationFunctionType
OP = mybir.AluOpType

B, S, D = 4, 2048, 1024
E, DFF = 16, 2048
CAP = 256
RH = 128          # router hidden
EL = 2            # experts per core
NCORES = 8
NTOK = B * S      # 8192
TPC = NTOK // NCORES  # tokens per core (1024)

DEBUG = bool(int(os.environ.get("MOE_KERNEL_DEBUG", "0")))


def _build_program():
    nc = bacc.Bacc(None, target_bir_lowering=False)

    hs = nc.dram_tensor("hs", [NTOK, D], BF16, kind="ExternalInput")
    # X^T for this core's 1024 tokens: [g, k, 128, 512]
    hstt = nc.dram_tensor("hstt", [2, 8, 128, 512], F32, kind="ExternalInput")
    r1t = nc.dram_tensor("r1t", [128, 8 * RH], F32, kind="ExternalInput")
    r2a = nc.dram_tensor("r2a", [RH, E], F32, kind="ExternalInput")
    w1t = nc.dram_tensor("w1t", [EL, 16, 128, 1024], BF16, kind="ExternalInput")
    w2t = nc.dram_tensor("w2t", [EL, 16, 128, 1024], BF16, kind="ExternalInput")
    w3c = nc.dram_tensor("w3c", [EL, DFF, D], BF16, kind="ExternalInput")
    ident = nc.dram_tensor("ident", [128, 128], F32, kind="ExternalInput")
    rep16 = nc.dram_tensor("rep16", [16, 128], F32, kind="ExternalInput")
    ones_1_16 = nc.dram_tensor("ones_1_16", [1, 16], F32, kind="ExternalInput")
    ones16_1 = nc.dram_tensor("ones16_1", [16, 1], F32, kind="ExternalInput")
    iota_w = nc.dram_tensor("iota_w", [16, 128], F32, kind="ExternalInput")

    outp = nc.dram_tensor("outp", [NTOK, D], F32, kind="ExternalOutput")
    if DEBUG:
        d_lT = nc.dram_tensor("d_lT", [EL, NTOK], F32, kind="ExternalOutput")
        d_tau = nc.dram_tensor("d_tau", [1, 16], F32, kind="ExternalOutput")
        d_nf = nc.dram_tensor("d_nf", [1, 16], mybir.dt.uint32, kind="ExternalOutput")
        d_idx = nc.dram_tensor("d_idx", [128, 128], mybir.dt.int16, kind="ExternalOutput")
        d_wpp = nc.dram_tensor("d_wpp", [128, 16], F32, kind="ExternalOutput")

    with TileContext(nc) as tc:
        with (
            tc.tile_pool(name="const", bufs=1) as cpool,
            tc.tile_pool(name="persist", bufs=1) as ppool,
            tc.tile_pool(name="dram", bufs=1, space="DRAM") as dpool,
        ):
            # router consts first: they gate the first matmul.  r1t's first
            # half (chunks k0..3) gates the very first matmul, so it loads
            # alone; the second half follows the first X^T chunks.  r2a
            # rides the Act queue.
            c_r1t = cpool.tile([128, 8 * RH], F32)
            nc.sync.dma_start(out=c_r1t[:, 0:512], in_=r1t[:, 0:512])
            c_r2a = cpool.tile([RH, E], F32)
            nc.scalar.dma_start(out=c_r2a, in_=r2a[:, :])

            c_rep16 = cpool.tile([16, 128], F32)
            c_o116 = cpool.tile([1, 16], F32)
            c_o161 = cpool.tile([16, 1], F32)
            c_iota = cpool.tile([16, 128], F32)

            def load_chain_consts():
                nc.sync.dma_start(out=c_rep16, in_=rep16[:, :])
                nc.sync.dma_start(out=c_o116, in_=ones_1_16[:, :])
                nc.sync.dma_start(out=c_o161, in_=ones16_1[:, :])
                nc.sync.dma_start(out=c_iota, in_=iota_w[:, :])

            p_idx16 = ppool.tile([128, 128], mybir.dt.int16)
            p_wpp = ppool.tile([128, 16], F32)
            p_cand2 = ppool.tile([16, 1024], F32)
            p_wself = ppool.tile([16, 160], F32)
            p_nfw = ppool.tile([1, 16], mybir.dt.uint32)

            lg_in = dpool.tile([E, TPC], F32)            # my tokens x 16 experts
            lg_out = dpool.tile([E, TPC], F32)           # my 2 experts x all tokens

            with (
                tc.tile_pool(name="xgb", bufs=1) as xgbpool,
            ):
                # per-(expert,batch) gathered tokens, transposed layout:
                # xgb[ei][b][p, k, t] = X[token_t, 128k + p] (bf16)
                xgb = [[None] * B for _ in range(EL)]
                for ei in range(EL):
                    for b in range(B):
                        xgb[ei][b] = xgbpool.tile(
                            [128, 8, CAP], BF16, name=f"xgb{ei}{b}",
                            tag=f"xgb{ei}{b}")

                # ------------- sharded router (plain fp32) -------------
                with (
                    tc.tile_pool(name="r_sb", bufs=1) as rpool,
                    tc.tile_pool(name="r_xts", bufs=16) as xtspool,
                    tc.tile_pool(name="r_ps", bufs=2, space="PSUM") as rps,
                    tc.tile_pool(name="r_pslg", bufs=1, space="PSUM") as pslg,
                ):
                    preS = rpool.tile([128, TPC], F32)
                    lgsb = rpool.tile([E, TPC], F32)
                    ps_lg = pslg.tile([E, TPC], F32)
                    # PE p-state warm-up: the tensor engine ramps 0.65 ->
                    # 2.4GHz only after ~3us of continuous work.  Burn the
                    # initial DMA-wait window on throwaway matmuls so the
                    # router runs at full clock.
                    warm = rpool.tile([128, 128], F32, name="warm")
                    nc.vector.memset(warm, 0.0)
                    # preload the Copy/Exp act-table sets during the idle
                    # head so the first chain tau-copy doesn't eat a
                    # 1.3us LoadActFuncSet mid-critical-path
                    warm_act = rpool.tile([1, 16], F32, name="warm_act")
                    nc.scalar.activation(warm_act, warm[0:1, 0:16], AF.Copy)
                    nc.scalar.activation(warm_act, warm[0:1, 0:16], AF.Exp)
                    nc.scalar.activation(warm_act, warm[0:1, 0:16], AF.Copy)
                    ps_warm = rps.tile([128, 128], F32, tag="warm")
                    for _w in range(7):
                        nc.tensor.matmul(ps_warm, warm, warm,
                                         start=True, stop=True)
                    # X^T streams on the sync queue, one chunk per DMA;
                    # r1t's second half slots in behind the first two
                    # chunks.
                    # first layer runs per 256-token half so each half's
                    # silu -> second layer -> drain pipelines behind the
                    # next half's accumulation; only the final half's chain
                    # is serial with the collective start.
                    for g in range(2):
                        xks = []
                        for k in range(8):
                            xk = xtspool.tile([128, 512], F32, tag="xts")
                            nc.sync.dma_start(out=xk, in_=hstt[g, k, :, :])
                            if g == 0 and k == 3:
                                nc.sync.dma_start(out=c_r1t[:, 512:1024],
                                                  in_=r1t[:, 512:1024])
                            xks.append(xk)
                        for c in range(2):
                            # each 256-token half accumulates in its own
                            # one-bank PSUM tile (cols 0:256 of a [128,512]
                            # tile) so the start-zeroing of one half can't
                            # clobber the other
                            ps_pre = rps.tile([128, 512], F32, tag="pspre")
                            hcols = slice(0, 256)
                            for k in range(8):
                                nc.tensor.matmul(
                                    ps_pre[:, hcols],
                                    c_r1t[:, 128 * k:128 * (k + 1)],
                                    xks[k][:, 256 * c:256 * (c + 1)],
                                    start=(k == 0), stop=(k == 7))
                            pcols = slice(512 * g + 256 * c,
                                          512 * g + 256 * (c + 1))
                            nc.scalar.activation(
                                preS[:, pcols], ps_pre[:, hcols], AF.Silu)
                            nc.tensor.matmul(ps_lg[:, pcols],
                                             c_r2a,
                                             preS[:, pcols],
                                             start=True, stop=True)
                            nc.vector.tensor_copy(lgsb[:, pcols],
                                                  ps_lg[:, pcols])
                            nc.sync.dma_start(
                                out=lg_in[:, pcols], in_=lgsb[:, pcols])

                load_chain_consts()

                # Shared expert pools: ONE PSUM tag-ring (bufs=3 slots of
                # [128,1024] = 6 banks) covers mm1-g, mm2-v, and mm3-o tiles
                # for BOTH experts, so phase/expert transitions never wait on
                # a pool handoff -- each new tile waits only on its own ring
                # slot's readers, which drained ~2 m-iterations earlier.  One
                # weight ring likewise streams w1/w2 for both experts; e1's
                # m=0 tiles issue (and land) during e0's mm3 automatically.
                eps_stack = contextlib.ExitStack()
                epspool = eps_stack.enter_context(
                    tc.tile_pool(name="e_ps", bufs=3, space="PSUM"))
                wm_stack = contextlib.ExitStack()
                wmpool = wm_stack.enter_context(
                    tc.tile_pool(name="e_wm", bufs=12))
                orow_stack = contextlib.ExitStack()
                orowpool = orow_stack.enter_context(
                    tc.tile_pool(name="e_orow", bufs=2))
                pre_w = []
                for m in range(4):
                    w1p = wmpool.tile([128, 1024], BF16, tag="wm")
                    nc.sync.dma_start(out=w1p, in_=w1t[0, m, :, :])
                    w2p = wmpool.tile([128, 1024], BF16, tag="wm")
                    nc.sync.dma_start(out=w2p, in_=w2t[0, m, :, :])
                    pre_w.append((w1p, w2p))

                nc.gpsimd.collective_compute(
                    "AllToAll",
                    mybir.AluOpType.bypass,
                    replica_groups=[list(range(NCORES))],
                    ins=[lg_in.opt()],
                    outs=[lg_out.opt()],
                )
                # lg_out[2j + ei, t] = logit(expert 2*me+ei, token 1024j + t)

                # ---------------- topk + dispatch chains ----------------
                tp_stack = contextlib.ExitStack()
                tpool = tp_stack.enter_context(
                    tc.tile_pool(name="t_sb", bufs=3))
                tspool = tp_stack.enter_context(
                    tc.tile_pool(name="t_sb1", bufs=1))
                psm_stack = contextlib.ExitStack()
                psmall = psm_stack.enter_context(
                    tc.tile_pool(name="t_ps", bufs=1, space="PSUM"))

                tau8 = tspool.tile([1, 16], F32)
                tau16 = tspool.tile([16, 8], F32)
                idxf = tspool.tile([16, 160], F32)
                nf = tspool.tile([1, 16], mybir.dt.uint32)
                qq = 1.0 - 255.5 / 2047.0

                # all 8 chains' lane-wrapped logit views, ONE DMA per
                # (expert, kind): partition maps to a column-group of lg_out
                # and the row-half index a rides in the free dim, so every
                # view dim maps to a single source dim (rearrange-legal).
                # lwq_all[p, (e b), (a j)] = lg_out[4b+2a+e, 8p+j]
                # lwr_all[p, (e b), (a f)] = lg_out[4b+2a+e, 64p+f]
                lwq_all = tspool.tile([128, 8, 16], F32)
                lwr_all = tspool.tile([16, 8, 128], F32)
                lgv = lg_out[:, :].rearrange(
                    "(b a e) (p j) -> e p b a j", b=4, a=2, e=2, p=128, j=8)
                lgw = lg_out[:, :].rearrange(
                    "(b a e) (p f) -> e p b a f", b=4, a=2, e=2, p=16, f=64)
                # lane-wrap DMAs go out on the Act queue: the sync queue
                # would head-of-line-block behind their collective wait,
                # stalling the w1/w2 prefetch stream that should run during
                # the collective window.  e0's kth input (lwq) issues first;
                # e1's views load inside emit_e1_chains so they don't sit
                # ahead of e0's chain Act ops in the queue.
                def load_lw_views(ei):
                    nc.scalar.dma_start(
                        out=lwq_all[:, 4 * ei:4 * (ei + 1), :].rearrange(
                            "p b (a j) -> p b a j", a=2, j=8),
                        in_=lgv[ei])
                    nc.scalar.dma_start(
                        out=lwr_all[:, 4 * ei:4 * (ei + 1), :].rearrange(
                            "p b (a f) -> p b a f", a=2, f=64),
                        in_=lgw[ei])

                load_lw_views(0)

                def chain_kth(b, ei):
                    q = 4 * ei + b
                    lwq = lwq_all[:, 4 * ei + b, :]
                    nc.gpsimd.kth_largest(tau8[0:1, 2 * q:2 * q + 2],
                                          lwq, n_per_lane=16, k=300,
                                          quantile=qq)

                cmp_tiles = [None] * 8

                def chain_sel(b, ei):
                    """topk select for (ei, b): threshold bcast, mask,
                    weights, and the sparse_gather of winner ids."""
                    q = 4 * ei + b
                    lwrap = lwr_all[:, 4 * ei + b, :]

                    ps_tau = psmall.tile([16, 1], F32, tag="pst")
                    nc.tensor.matmul(ps_tau, c_o116,
                                     tau8[0:1, 2 * q + 1:2 * q + 2],
                                     start=True, stop=True)
                    nc.scalar.activation(tau16[:, q:q + 1], ps_tau, AF.Copy)

                    t16b = tau16[:, q:q + 1].to_broadcast([16, 128])
                    cmp = tpool.tile([16, 128], mybir.dt.uint8, tag="cmp",
                                     bufs=8)
                    cmp_tiles[q] = cmp
                    nc.vector.tensor_tensor(cmp, lwrap, t16b, OP.is_gt)
                    cand = tpool.tile([16, 128], F32, tag="cand")
                    nc.vector.memset(cand, -1.0)
                    nc.vector.copy_predicated(cand, cmp, c_iota)

                    nc.gpsimd.sparse_gather(idxf[:, 20 * q:20 * q + 20],
                                            cand,
                                            num_found=nf[0:1, q:q + 1])

                def chain_wsel(b, ei):
                    """wpp-only mask bookkeeping, deferred off the chain
                    critical path: exp(logit - tau) masked into p_cand2."""
                    q = 4 * ei + b
                    lwrap = lwr_all[:, 4 * ei + b, :]
                    t16b = tau16[:, q:q + 1].to_broadcast([16, 128])
                    cmp = cmp_tiles[q]
                    esub = tpool.tile([16, 128], F32, tag="esub")
                    nc.vector.tensor_tensor(esub, lwrap, t16b, OP.subtract)
                    eexp = tpool.tile([16, 128], F32, tag="eexp")
                    nc.scalar.activation(eexp, esub, AF.Exp)
                    nc.vector.memset(p_cand2[:, 128 * q:128 * (q + 1)], -1.0)
                    nc.vector.copy_predicated(
                        p_cand2[:, 128 * q:128 * (q + 1)], cmp, eexp)

                def chain_gather(b, ei):
                    """idx replication + token gather for (ei, b)."""
                    q = 4 * ei + b
                    ps_idx = psmall.tile([128, 16], F32, tag="psi")
                    nc.tensor.matmul(ps_idx, c_rep16,
                                     idxf[:, 20 * q:20 * q + 16],
                                     start=True, stop=True)
                    nc.vector.tensor_copy(p_idx16[:, 16 * q:16 * (q + 1)],
                                          ps_idx)

                    nc.gpsimd.dma_gather(
                        xgb[ei][b][:, :, :], hs[2048 * b:2048 * (b + 1), :],
                        p_idx16[:, 16 * q:16 * (q + 1)],
                        num_idxs=CAP, num_idxs_reg=CAP, elem_size=D,
                        transpose=True)

                # chain-major interleave: b0's gather issues as early as
                # possible; later chains' kth/select ride in the latency
                # windows of the earlier chains' Pool ops.
                chain_kth(0, 0)
                chain_sel(0, 0)
                chain_kth(1, 0)
                chain_gather(0, 0)
                chain_sel(1, 0)
                chain_kth(2, 0)
                chain_gather(1, 0)
                chain_sel(2, 0)
                chain_kth(3, 0)
                chain_gather(2, 0)
                chain_sel(3, 0)
                chain_gather(3, 0)

                def emit_e1_kth():
                    load_lw_views(1)
                    for b in range(B):
                        chain_kth(b, 1)

                def emit_e1_sel():
                    for b in range(B):
                        chain_sel(b, 1)

                def emit_e1_gather():
                    for b in range(B):
                        chain_gather(b, 1)

                if DEBUG:
                    dl_v = d_lT[:, :].rearrange(
                        "e (j t) -> e j t", j=8, t=TPC)
                    lo_v = lg_out[:, :].rearrange(
                        "(j e) t -> e j t", j=8, e=2)
                    nc.sync.dma_start(out=dl_v, in_=lo_v)
                    nc.sync.dma_start(out=d_tau[:, :], in_=tau8)
                    nc.sync.dma_start(out=d_nf[:, 0:8], in_=nf[:, 0:8])
                    nc.sync.dma_start(out=d_idx[:, :], in_=p_idx16)


                # ---- gating-weight normalization (off critical path) ----
                def emit_wpp_chain():
                    if True:
                        wps = psmall
                        for q in range(8):
                            nc.gpsimd.sparse_gather(
                                p_wself[:, 20 * q:20 * q + 20],
                                p_cand2[:, 128 * q:128 * (q + 1)],
                                num_found=p_nfw[0:1, q:q + 1])
                        wsel_r = p_wself[:, :].rearrange(
                            "p (q x) -> p q x", q=8)[:, :, 0:16]
                        ps_sum = wps.tile([1, 128], F32, tag="pst")
                        nc.tensor.matmul(ps_sum, c_o161, wsel_r,
                                         start=True, stop=True)
                        sums = ppool.tile([1, 8], F32, tag="sums")
                        nc.vector.tensor_reduce(
                            sums,
                            ps_sum[:, :].rearrange("p (q x) -> p q x", q=8),
                            mybir.AxisListType.X, OP.add)
                        nc.vector.tensor_scalar_add(sums, sums, 1e-9)
                        rec = ppool.tile([1, 8], F32, tag="rec")
                        nc.vector.reciprocal(rec, sums)
                        ps_rec16 = wps.tile([16, 8], F32, tag="psi")
                        nc.tensor.matmul(ps_rec16, c_o116, rec,
                                         start=True, stop=True)
                        rec16 = ppool.tile([16, 8], F32, tag="rec16")
                        nc.scalar.activation(rec16, ps_rec16, AF.Copy)
                        wnorm = ppool.tile([16, 160], F32, tag="wnorm")
                        wn_r = wnorm[:, :].rearrange("p (q x) -> p q x", q=8)
                        ws_r = p_wself[:, :].rearrange("p (q x) -> p q x", q=8)
                        nc.vector.tensor_tensor(
                            wn_r, ws_r,
                            rec16[:, :].to_broadcast([16, 8, 20]), OP.mult)
                        wn_sx = wnorm[:, :].rearrange("p (q x) -> p x q", q=8)
                        for g in range(8):
                            for s in range(2):
                                dstap = p_wpp[16 * g:16 * (g + 1), :].rearrange(
                                    "p (q s) -> p s q", q=8, s=2)[:, s, :]
                                nc.gpsimd.dma_start(out=dstap,
                                                    in_=wn_sx[:, 8 * s + g, :])
                        if DEBUG:
                            nc.sync.dma_start(out=d_wpp[:, :], in_=p_wpp)

                # ---------------- experts ----------------
                for ei in range(EL):
                    ex_stack = contextlib.ExitStack()
                    exw3_stack = contextlib.ExitStack()
                    hpool = ex_stack.enter_context(
                        tc.tile_pool(name=f"e{ei}_h", bufs=16))
                    w3pool = exw3_stack.enter_context(
                        tc.tile_pool(name=f"e{ei}_w3r", bufs=1))
                    # resident W3 tiles: loads interleaved into the m-loop
                    # below so they ride the mm1/mm2 DMA slack without
                    # delaying the W1/W2 stream
                    w3res = []
                    h_tiles = []
                    def get_w(m):
                        plist = pre_w if ei == 0 else []
                        if m < len(plist):
                            return plist[m]
                        w1m = wmpool.tile([128, 1024], BF16, tag="wm")
                        nc.sync.dma_start(out=w1m, in_=w1t[ei, m, :, :])
                        w2m = wmpool.tile([128, 1024], BF16, tag="wm")
                        nc.sync.dma_start(out=w2m, in_=w2t[ei, m, :, :])
                        return w1m, w2m

                    # PSUM start zeroes a whole 2KB bank, so the two
                    # 256-col token blocks sharing a bank form ONE
                    # accumulation group (start on first, stop on last).
                    def mm_batch(ps, wt, b, bl):
                        for k in range(8):
                            nc.tensor.matmul(
                                ps[:, 256 * b:256 * (b + 1)],
                                wt[:, 128 * k:128 * (k + 1)],
                                xgb[ei][b][:, k, :],
                                start=(k == 0 and bl == 0),
                                stop=(k == 7 and bl == 1),
                                skip_group_check=True)

                    if True:
                        for m in range(16):
                            if m == 1 and ei == 0:
                                continue  # fused into m == 0 below
                            if m == 0 and ei == 0:
                                # First two m-iterations interleaved
                                # batch-major: each token gather unblocks a
                                # 24-matmul (2.6us) run, matching the Pool
                                # engine's ~2.5us per-chain dispatch cadence
                                # so the PE never starves while the four
                                # gathers trickle in.
                                w1m0, w2m0 = get_w(0)
                                w1m1, w2m1 = get_w(1)
                                ps_g0 = epspool.tile([128, 1024], F32,
                                                     tag="ps")
                                ps_v0 = epspool.tile([128, 1024], F32,
                                                     tag="ps")
                                ps_g1 = epspool.tile([128, 1024], F32,
                                                     tag="ps")
                                for bh in range(2):
                                    for bl in range(2):
                                        b = 2 * bh + bl
                                        mm_batch(ps_g0, w1m0, b, bl)
                                        mm_batch(ps_v0, w2m0, b, bl)
                                        mm_batch(ps_g1, w1m1, b, bl)
                                hm0 = hpool.tile([128, 1024], BF16, tag="h")
                                nc.scalar.activation(hm0, ps_g0, AF.Silu)
                                ps_v1 = epspool.tile([128, 1024], F32,
                                                     tag="ps")
                                for bh in range(2):
                                    for bl in range(2):
                                        mm_batch(ps_v1, w2m1, 2 * bh + bl, bl)
                                nc.vector.tensor_mul(hm0, hm0, ps_v0)
                                hm1 = hpool.tile([128, 1024], BF16, tag="h")
                                nc.scalar.activation(hm1, ps_g1, AF.Silu)
                                nc.vector.tensor_mul(hm1, hm1, ps_v1)
                                h_tiles.append(hm0)
                                h_tiles.append(hm1)
                                continue
                            w1m, w2m = get_w(m)
                            ps_g = epspool.tile([128, 1024], F32, tag="ps")
                            ps_v = epspool.tile([128, 1024], F32, tag="ps")
                            for bh in range(2):
                                for bl in range(2):
                                    b = 2 * bh + bl
                                    mm_batch(ps_g, w1m, b, bl)
                                    mm_batch(ps_v, w2m, b, bl)
                            hm = hpool.tile([128, 1024], BF16, tag="h")
                            nc.scalar.activation(hm, ps_g, AF.Silu)
                            nc.vector.tensor_mul(hm, hm, ps_v)
                            h_tiles.append(hm)
                            # w3 tiles have no input deps, so if emitted at
                            # iteration m they'd fire immediately and flood
                            # the DMA engines during the collective/gather
                            # window.  Emit w3[m-5] at m: the sync queue is
                            # then blocked behind m4's w1/w2 load (which
                            # waits on m0's matmuls), so the w3 stream only
                            # starts once the expert phase is underway.
                            def load_w3(mm):
                                w3k = w3pool.tile(
                                    [128, 1024], BF16,
                                    name=f"w3r{ei}{mm}", tag=f"w3r{mm}")
                                # dep-free loads get hoisted by the Tile
                                # scheduler into the collective/gather
                                # window, starving the latency-critical lw
                                # views and token gathers of DMA bandwidth;
                                # floor them past that window (e1's are
                                # naturally gated by e0's pool close).
                                with tc.tile_wait_until(62e-3, enable=(ei == 0)):
                                    nc.sync.dma_start(
                                        out=w3k,
                                        in_=w3c[ei, 128 * mm:128 * (mm + 1), :])
                                w3res.append(w3k)
                            if m >= 5:
                                load_w3(m - 5)
                            if m == 15:
                                for mm in range(11, 16):
                                    load_w3(mm)
                            if ei == 0 and m == 4:
                                emit_e1_kth()
                            if ei == 0 and m == 6:
                                emit_e1_sel()
                            if ei == 0 and m == 8:
                                emit_e1_gather()
                            if ei == 0 and m == 10:
                                for _b in range(B):
                                    chain_wsel(_b, 0)
                                    chain_wsel(_b, 1)
                                emit_wpp_chain()
                    # mm3: batch-outer over resident W3; each batch's
                    # scatter fires as soon as its rows drain, so only the
                    # last batch's scatter is exposed at the end
                    for b in range(B):
                        for s in range(2):
                            orow_t = orowpool.tile(
                                [128, 1, 1024], F32,
                                name=f"orow{ei}{b}{s}", tag="or")
                            col = 8 * ei + 2 * b + s
                            q = 4 * ei + b
                            # drain + scatter each 512-col (PSUM-bank) half
                            # as soon as its accumulation group stops, on
                            # alternating engines: the dh0 drain overlaps
                            # dh1's matmuls, halving the exposed tail chain
                            # after the final block.  Each half gets its OWN
                            # PSUM tile so dh1's matmuls don't WAR-wait on
                            # dh0's drain read.
                            last = False
                            for dh in range(2):
                                ps_o = epspool.tile([128, 512], F32,
                                                    tag="ps")
                                for k in range(16):
                                    nc.tensor.matmul(
                                        ps_o,
                                        h_tiles[k][:, 256 * b + 128 * s:
                                                   256 * b + 128 * (s + 1)],
                                        w3res[k][:, 512 * dh:512 * (dh + 1)],
                                        start=(k == 0), stop=(k == 15))
                                # the very last half drains in two 256-col
                                # quarters so only a quarter's act+prep+DMA
                                # chain is exposed after the final matmul
                                nq = 2 if (last and dh == 1) else 1
                                for qi in range(nq):
                                    qw = 512 // nq
                                    part = slice(
                                        512 * dh + qw * qi,
                                        512 * dh + qw * (qi + 1))
                                    dst = orow_t[:, 0, part]
                                    psq = ps_o[:, qw * qi:qw * (qi + 1)]
                                    if (s + dh + qi) % 2 == 0:
                                        nc.vector.tensor_scalar(
                                            dst, psq,
                                            p_wpp[:, col:col + 1], None,
                                            op0=OP.mult)
                                    else:
                                        nc.scalar.activation(
                                            dst, psq, AF.Copy,
                                            scale=p_wpp[:, col:col + 1])
                                    # token slots t of chunk (b, s) live in
                                    # idx columns 8s..8s+8 (t <-> idx[t%16,
                                    # t//16])
                                    nc.gpsimd.dma_scatter_add(
                                        outp[2048 * b:2048 * (b + 1), part],
                                        orow_t[:, :, part],
                                        p_idx16[:, 16 * q + 8 * s:
                                                16 * q + 8 * (s + 1)],
                                        num_idxs=128, num_idxs_reg=128,
                                        elem_size=qw, elem_step=D)
                    exw3_stack.close()
                    ex_stack.close()
                psm_stack.close()
                tp_stack.close()
                orow_stack.close()
                wm_stack.close()
                eps_stack.close()

    nc.finalize()
    return nc


_PROGRAM = None


def _get_program():
    global _PROGRAM
    if _PROGRAM is None:
        _PROGRAM = _build_program()
    return _PROGRAM


def _host_inputs(hidden_states, router_w1, router_w2, w1, w2, w3):
    """Builds per-core in_maps (host-side slicing / retiling)."""
    import ml_dtypes
    hs32 = np.ascontiguousarray(hidden_states.reshape(NTOK, D)).astype(np.float32)
    hs = hs32.astype(ml_dtypes.bfloat16)
    r1t = np.ascontiguousarray(
        np.asarray(router_w1, np.float32).reshape(8, 128, RH).transpose(1, 0, 2)
    ).reshape(128, 8 * RH)
    r2a = np.ascontiguousarray(np.asarray(router_w2, np.float32))
    ident = np.eye(128, dtype=np.float32)
    rep16 = np.zeros((16, 128), np.float32)
    for m in range(128):
        rep16[m % 16, m] = 1.0
    ones_1_16 = np.ones((1, 16), np.float32)
    ones16_1 = np.ones((16, 1), np.float32)
    _rows = np.arange(16, dtype=np.float32)[:, None]
    _cols = np.arange(128, dtype=np.float32)[None, :]
    iota_w = 1024.0 * np.floor(_cols / 64.0) + 64.0 * _rows + np.mod(_cols, 64.0)

    def tile_w(we):  # [D, DFF] -> [16, 128, 1024]
        return np.ascontiguousarray(
            we.reshape(8, 128, 16, 128).transpose(2, 1, 0, 3)).reshape(16, 128, 1024)

    w1 = np.asarray(w1, np.float32)
    w2 = np.asarray(w2, np.float32)
    w3 = np.asarray(w3, np.float32)

    hsT = np.ascontiguousarray(hs32.T)  # [D, NTOK]
    # per-core X^T shard: [core][g=2, k=8, 128, 512]
    hstt_all = np.ascontiguousarray(
        hsT.reshape(8, 128, NCORES, 2, 512).transpose(2, 3, 0, 1, 4))
    in_maps = []
    for c in range(NCORES):
        e0 = EL * c
        w1c = np.stack([tile_w(w1[e0 + j]) for j in range(EL)]).astype(
            ml_dtypes.bfloat16)
        w2c = np.stack([tile_w(w2[e0 + j]) for j in range(EL)]).astype(
            ml_dtypes.bfloat16)
        w3cc = np.ascontiguousarray(w3[e0:e0 + EL]).astype(ml_dtypes.bfloat16)
        in_maps.append({
            "hs": hs, "hstt": hstt_all[c],
            "r1t": r1t, "r2a": r2a,
            "w1t": w1c, "w2t": w2c, "w3c": w3cc,
            "ident": ident, "rep16": rep16,
            "ones_1_16": ones_1_16, "ones16_1": ones16_1, "iota_w": iota_w,
        })
    return in_maps


_LAST_RESULTS = None  # for test introspection


def kernel(hidden_states, router_w1, router_w2, w1, w2, w3):
    global _LAST_RESULTS
    nc = _get_program()
    in_maps = _host_inputs(hidden_states, router_w1, router_w2, w1, w2, w3)
    trace = bool(int(os.environ.get("MOE_KERNEL_TRACE", "0")))
    res = run_bass_kernel_spmd(nc, in_maps, core_ids=list(range(NCORES)), trace=trace)
    _LAST_RESULTS = res
    out = np.zeros((NTOK, D), np.float32)
    for r in res.results:
        out += r["outp"]
    return out.reshape(B, S, D)

